# revision 24
# baseline (speedup 1.0000x reference)
"""Causal self-attention + cross-attention Trainium2 kernel (8 NeuronCores).

Sharding: head-parallel. 16 heads x 2 batches = 32 (b,h) pairs; core c owns
heads {2c, 2c+1} for both batches (its 128 channels of C=1024). Projections
are column-sliced per core; attention runs fully local per head; the output
projection is row-sliced and the 8 partial [B*T, C] fp16 outputs are summed
on the host (no device collectives).

All matmuls are fp16 (1 cycle/row on PE, fp32 PSUM accumulate). Softmax
without max-subtraction (scores bounded ~|8| here), exp on ScalarE with the
1/sqrt(D) scale folded in, scores computed transposed (ST[k,q]) so no
probability transpose is needed before AV. V tiles carry a ones column so
AV row 64 accumulates the softmax denominator.

The whole kernel is a single software-pipelined phase sequence emitted as
  cross-proj(b0), proj(ch0), attn(b0,q0)+out-proj, proj(ch1), attn(b0,q1)...
so projection matmuls fill the PE whenever attention stalls on ScalarE's
exp, the output projection overlaps attention instead of running cold at
the tail, and the PE never idles long enough for HAM to re-throttle.
V is produced token-major directly (x-tile stationary, Wv moving), which
removes the PE-transpose pass and frees its PSUM bank; PSUM budget is
proj/out 2x[128,512] + scores 2x[128,1024] + 2 AV accumulators = 8 banks.
"""
import sys

sys.path.insert(0, "/opt/trn_rl_repo")

import numpy as np

import concourse.bass as bass
import concourse.tile as tile
from concourse import bacc, mybir
from concourse.bass_utils import run_bass_kernel_spmd

dt = mybir.dt

B, T, TC, C, CC, H, D = 2, 2048, 512, 1024, 512, 16, 64
NCORES = 8
CPC = 128          # channels per core = 2 heads * 64
NT = B * T         # 4096 tokens (batch-major)
NTC = B * TC       # 1024 cross tokens
KT_X = C // 128    # 8 contraction tiles over C
KT_C = CC // 128   # 4 contraction tiles over CC
NCH = NT // 512    # 8 token chunks
NCHC = NTC // 512  # 2 cross token chunks
QC_PER_B = T // 512  # 4 q-chunks per batch
KT_PER_B = T // 128  # 16 k-tiles per batch
LOOKAHEAD = 2      # kt steps issued ahead of their AV in the PE queue


def _build(zero_bias=False):
    f32, f16 = dt.float32, dt.float16
    nc = bacc.Bacc("TRN2", target_bir_lowering=False, debug=False,
                   enable_asserts=True, num_devices=NCORES)

    xT = nc.dram_tensor("xT", [NCH, 128, KT_X, 512], f16, kind="ExternalInput").ap()
    cT = nc.dram_tensor("cT", [NCHC, 128, KT_C, 512], f16, kind="ExternalInput").ap()
    wq = nc.dram_tensor("wq", [128, KT_X, CPC], f16, kind="ExternalInput").ap()
    wk = nc.dram_tensor("wk", [128, KT_X, CPC], f16, kind="ExternalInput").ap()
    wv = nc.dram_tensor("wv", [128, KT_X, CPC], f16, kind="ExternalInput").ap()
    wcq = nc.dram_tensor("wcq", [128, KT_X, CPC], f16, kind="ExternalInput").ap()
    wck = nc.dram_tensor("wck", [128, KT_C, CPC], f16, kind="ExternalInput").ap()
    wcv = nc.dram_tensor("wcv", [128, KT_C, CPC], f16, kind="ExternalInput").ap()
    wp = nc.dram_tensor("wp", [CPC, C], f16, kind="ExternalInput").ap()
    bias8 = nc.dram_tensor("bias8", [CPC, 8], f32, kind="ExternalInput").ap()
    maskd = nc.dram_tensor("mask2", [128, 256], f16, kind="ExternalInput").ap()
    onesd = nc.dram_tensor("ones64", [128, 64], dt.float32r, kind="ExternalInput").ap()
    out = nc.dram_tensor("out", [NT, C], f16, kind="ExternalOutput").ap()

    Exp = mybir.ActivationFunctionType.Exp
    SCALE = 0.125  # 1/sqrt(D)

    with tile.TileContext(nc) as tc:
        from contextlib import ExitStack
        with ExitStack() as es:
            persist = es.enter_context(tc.tile_pool(name="persist", bufs=1))
            qT_t = persist.tile([128, NT], f16, tag="qT")
            kT_t = persist.tile([128, NT], f16, tag="kT")
            qcT_t = persist.tile([128, NT], f16, tag="qcT")
            kcT_t = persist.tile([128, NTC], f16, tag="kcT")
            vn_t = persist.tile([128, (NT // 128) * 130], f16, tag="vn")
            vcn_t = persist.tile([128, (NTC // 128) * 130], f16, tag="vcn")
            yT2_t = persist.tile([128, NT], f16, tag="yT2")
            wp_t = persist.tile([128, C], f16, tag="wp")
            bias_t = persist.tile([128, 8], f32, tag="bias")
            mask_t = persist.tile([128, 256], f16, tag="mask")

            wq_t = persist.tile([128, KT_X, CPC], f16, tag="wq")
            wk_t = persist.tile([128, KT_X, CPC], f16, tag="wk")
            wv_t = persist.tile([128, KT_X, CPC], f16, tag="wv")
            wcq_t = persist.tile([128, KT_X, CPC], f16, tag="wcq")
            wck_t = persist.tile([128, KT_C, CPC], f16, tag="wck")
            wcv_t = persist.tile([128, KT_C, CPC], f16, tag="wcv")

            # critical-path loads first, split across engine DMA queues:
            # scalar carries weights, sync carries x/cross chunks (in-loop),
            # gpsimd/vector carry tensors not needed until later.
            for wdram, wtile in ((wck, wck_t), (wcv, wcv_t), (wq, wq_t),
                                 (wk, wk_t), (wv, wv_t), (wcq, wcq_t)):
                nc.scalar.dma_start(out=wtile[:], in_=wdram[:])
            nc.gpsimd.dma_start(out=bias_t[:], in_=bias8[:])
            nc.gpsimd.dma_start(out=mask_t[:], in_=maskd[:])
            nc.gpsimd.dma_start(out=wp_t[:], in_=wp[:])

            vn_r = vn_t[:].rearrange("p (t c) -> p t c", c=130)
            nc.vector.memset(vn_r[:, :, 64:65], 1.0)
            nc.vector.memset(vn_r[:, :, 129:130], 1.0)
            vcn_r = vcn_t[:].rearrange("p (t c) -> p t c", c=130)
            nc.vector.memset(vcn_r[:, :, 64:65], 1.0)
            nc.vector.memset(vcn_r[:, :, 129:130], 1.0)
            ones_t = persist.tile([128, 64], dt.float32r, tag="ones")
            nc.gpsimd.dma_start(out=ones_t[:], in_=onesd[:])

            apool = es.enter_context(tc.tile_pool(name="apool", bufs=3))
            bpool = es.enter_context(tc.tile_pool(name="bpool", bufs=3))
            rdpool = es.enter_context(tc.tile_pool(name="rdpool", bufs=2, space="DRAM"))
            ypool = es.enter_context(tc.tile_pool(name="ypool", bufs=4))
            expool = es.enter_context(tc.tile_pool(name="expool", bufs=12))
            youtp = es.enter_context(tc.tile_pool(name="youtp", bufs=6))
            pspool = es.enter_context(tc.tile_pool(name="pspool", bufs=2, space="PSUM"))
            stps = es.enter_context(tc.tile_pool(name="stps", bufs=2, space="PSUM"))
            yhps = es.enter_context(tc.tile_pool(name="yhps", bufs=1, space="PSUM"))

            def evict(dst_slice, ps, bcol):
                if zero_bias:
                    nc.vector.tensor_copy(dst_slice, ps[:])
                else:
                    nc.vector.tensor_scalar_add(dst_slice, ps[:],
                                                bias_t[:, bcol:bcol + 1])

            xblks = {}

            def get_xblk(ch):
                if ch not in xblks:
                    xblk = apool.tile([128, KT_X, 512], f16, tag="xblk")
                    nc.sync.dma_start(out=xblk[:], in_=xT[ch])
                    xblks[ch] = xblk
                return xblks[ch]

            def chan_chain(ch, wtile, dst, bcol):
                """One channel-major projection chain for a 512-token chunk."""
                xblk = get_xblk(ch)
                ps = pspool.tile([128, 512], f32, tag="ps")
                for kt in range(KT_X):
                    nc.tensor.matmul(ps[:], wtile[:, kt, :], xblk[:, kt, :],
                                     start=(kt == 0), stop=(kt == KT_X - 1))
                evict(dst[:, ch * 512:(ch + 1) * 512], ps, bcol)

            def v_chain(ch):
                """v token-major: out[tok, d] accumulated per 128-tok subtile."""
                xblk = get_xblk(ch)
                pv = pspool.tile([128, 512], f32, tag="ps")
                for tt in range(4):
                    for kt in range(KT_X):
                        nc.tensor.matmul(
                            pv[:, tt * 128:(tt + 1) * 128],
                            xblk[:, kt, tt * 128:(tt + 1) * 128],
                            wv_t[:, kt, :],
                            start=(kt == 0), stop=(kt == KT_X - 1))
                # single strided copy into vn [v_h0 | 1 | v_h1 | 1] slots
                base = ch * 4 * 130
                dst = vn_t[:, base:base + 4 * 130].rearrange(
                    "p (t h c) -> p t h c", t=4, h=2)[:, :, :, 0:64]
                src = pv[:].rearrange("p (t h c) -> p t h c", t=4, h=2)
                nc.vector.tensor_copy(dst, src)

            def cross_chunk(chc):
                """kc projection (channel-major) + vc (token-major) for one
                512-token cross chunk."""
                cblk = apool.tile([128, KT_C, 512], f16, tag="cblk", bufs=2)
                nc.sync.dma_start(out=cblk[:], in_=cT[chc])
                ps = pspool.tile([128, 512], f32, tag="ps")
                for kt in range(KT_C):
                    nc.tensor.matmul(ps[:], wck_t[:, kt, :], cblk[:, kt, :],
                                     start=(kt == 0), stop=(kt == KT_C - 1))
                evict(kcT_t[:, chc * 512:(chc + 1) * 512], ps, 2)
                pv = pspool.tile([128, 512], f32, tag="ps")
                for tt in range(4):
                    for kt in range(KT_C):
                        nc.tensor.matmul(
                            pv[:, tt * 128:(tt + 1) * 128],
                            cblk[:, kt, tt * 128:(tt + 1) * 128],
                            wcv_t[:, kt, :],
                            start=(kt == 0), stop=(kt == KT_C - 1))
                base = chc * 4 * 130
                dst = vcn_t[:, base:base + 4 * 130].rearrange(
                    "p (t h c) -> p t h c", t=4, h=2)[:, :, :, 0:64]
                src = pv[:].rearrange("p (t h c) -> p t h c", t=4, h=2)
                nc.vector.tensor_copy(dst, src)

            mask_r = mask_t[:].rearrange("p (h q) -> p h q", h=2)

            def attn_part(b, qc, qlo, is_self, mid_hook=None):
                """One softmax-attention accumulation (self or cross) for a
                512-wide q chunk of batch b. Both heads' transposed score
                tiles live in one [128,1024] two-bank PSUM tile so a single
                exp serves both. Returns per-head normalized [64,512]."""
                nkt = (4 * qc + 4) if is_self else KT_C
                mid_kt = nkt // 2
                yh = yhps.tile([65, 1024], f32, tag="yh")

                pend = []
                fidx = [0]

                def flush_one():
                    ex, off, vsrc, vc0, vc1 = pend.pop(0)
                    first = fidx[0] == 0
                    last = fidx[0] == nkt - 1
                    fidx[0] += 1
                    nc.tensor.matmul(
                        yh[0:65, off:512],
                        vsrc[:, vc0:vc0 + 65],
                        ex[:, off:512],
                        start=first, stop=last)
                    nc.tensor.matmul(
                        yh[0:65, 512 + off:1024],
                        vsrc[:, vc1:vc1 + 65],
                        ex[:, 512 + off:1024],
                        start=first, stop=last)

                for kt in range(nkt):
                    if mid_hook is not None and kt == mid_kt:
                        mid_hook()
                    if is_self:
                        crossing = kt >= 4 * qc
                        off = (kt - 4 * qc) * 128 if crossing else 0
                        klo = b * T + kt * 128
                        ksrc, qsrc, vsrc = kT_t, qT_t, vn_t
                        vbase = (b * KT_PER_B + kt) * 130
                    else:
                        crossing, off = False, 0
                        klo = b * TC + kt * 128
                        ksrc, qsrc, vsrc = kcT_t, qcT_t, vcn_t
                        vbase = (b * KT_C + kt) * 130
                    st = stps.tile([128, 1024], f32, tag="st")
                    nc.tensor.matmul(
                        st[:, off:512],
                        ksrc[0:64, klo:klo + 128],
                        qsrc[0:64, qlo + off:qlo + 512],
                        start=True, stop=True)
                    nc.tensor.matmul(
                        st[:, 512 + off:1024],
                        ksrc[64:128, klo:klo + 128],
                        qsrc[64:128, qlo + off:qlo + 512],
                        start=True, stop=True)
                    ex = expool.tile([128, 1024], f16, tag="ex")
                    if off == 0:
                        nc.scalar.activation(ex[:], st[:], Exp, scale=SCALE)
                    else:
                        ex3 = ex[:].rearrange("p (h q) -> p h q", h=2)
                        st3 = st[:].rearrange("p (h q) -> p h q", h=2)
                        nc.scalar.activation(ex3[:, :, off:512],
                                             st3[:, :, off:512], Exp, scale=SCALE)
                    if crossing:
                        ex3 = ex[:].rearrange("p (h q) -> p h q", h=2)
                        nc.gpsimd.tensor_mul(ex3[:, :, off:off + 128],
                                             ex3[:, :, off:off + 128], mask_r)
                    pend.append((ex, off, vsrc, vbase, vbase + 65))
                    if len(pend) > LOOKAHEAD:
                        flush_one()
                while pend:
                    flush_one()

                def finish(tail=False):
                    # Evict accumulators (incl. the denominator row 64) to
                    # SBUF, get 1/denominator broadcast to 64 partitions,
                    # then one wide multiply on DVE. Mid-stream parts use a
                    # DMA bounce through DRAM (compact reciprocal on
                    # [128,8], broadcast-read back) — fully hidden behind
                    # ScalarE/PE. The tail-exposed final part instead
                    # broadcasts with two rank-1 f32r matmuls (PE is idle
                    # there) + fast approx reciprocal, which is ~6us
                    # shorter in latency.
                    ysb = bpool.tile([65, 1024], f32, tag="ysb")
                    nc.vector.tensor_copy(ysb[:], yh[:])
                    if tail:
                        dsr = bpool.tile([65, 1024], dt.float32r, tag="dsr")
                        nc.vector.tensor_copy(dsr[64:65, :], yh[64:65, :])
                        pbA = pspool.tile([64, 512], f32, tag="ps")
                        nc.tensor.matmul(pbA[:], ones_t[64:65, :],
                                         dsr[64:65, 0:512],
                                         start=True, stop=True)
                        pbB = pspool.tile([64, 512], f32, tag="ps")
                        nc.tensor.matmul(pbB[:], ones_t[64:65, :],
                                         dsr[64:65, 512:1024],
                                         start=True, stop=True)
                        r = bpool.tile([64, 1024], f32, tag="bc")
                        nc.vector.reciprocal_approx_fast(r[:, 0:512], pbA[:])
                        nc.vector.reciprocal_approx_fast(r[:, 512:1024],
                                                         pbB[:])
                    else:
                        drd = rdpool.tile([1, 1024], f32, tag="drd")
                        nc.sync.dma_start(out=drd[:], in_=ysb[64:65, :])
                        dsb = bpool.tile([128, 8], f32, tag="dsb")
                        nc.sync.dma_start(
                            out=dsb[:],
                            in_=drd[:].rearrange("a (p e) -> p (a e)", p=128))
                        rsb = bpool.tile([128, 8], f32, tag="rsb")
                        nc.vector.reciprocal(rsb[:], dsb[:])
                        rrd = rdpool.tile([1, 1024], f32, tag="rrd")
                        nc.sync.dma_start(
                            out=rrd[:].rearrange("a (p e) -> p (a e)", p=128),
                            in_=rsb[:])
                        r = bpool.tile([64, 1024], f32, tag="bc")
                        nc.sync.dma_start(
                            out=r[:, 0:512],
                            in_=rrd[0:1, 0:512].to_broadcast((64, 512)))
                        nc.sync.dma_start(
                            out=r[:, 512:1024],
                            in_=rrd[0:1, 512:1024].to_broadcast((64, 512)))
                    yab = ypool.tile([64, 1024], f32, tag="yab")
                    nc.vector.tensor_mul(yab[:], ysb[0:64, :], r[:])
                    return yab

                return finish

            def combine(b, qc, qlo, y_s, y_c):
                if zero_bias:
                    nc.vector.tensor_add(yT2_t[0:64, qlo:qlo + 512],
                                         y_s[:, 0:512], y_c[:, 0:512])
                    ybsum = ypool.tile([64, 512], f16, tag="ybsum")
                    nc.vector.tensor_add(ybsum[:], y_s[:, 512:1024],
                                         y_c[:, 512:1024])
                else:
                    add = mybir.AluOpType.add
                    nc.vector.scalar_tensor_tensor(
                        yT2_t[0:64, qlo:qlo + 512], y_s[:, 0:512],
                        bias_t[0:64, 4:5], y_c[:, 0:512], add, add)
                    ybsum = ypool.tile([64, 512], f16, tag="ybsum")
                    nc.vector.scalar_tensor_tensor(
                        ybsum[:], y_s[:, 512:1024], bias_t[0:64, 5:6],
                        y_c[:, 512:1024], add, add)
                # partition shift rows 0-63 -> 64-127 via SBUF-SBUF DMA
                nc.sync.dma_start(out=yT2_t[64:128, qlo:qlo + 512],
                                  in_=ybsum[:])

            def out_proj(qlo):
                """Output projection for one 512-token chunk of yT2.
                Evictions mostly on DVE; 1 in 4 on ScalarE (which has slack)."""
                for tt in range(4):
                    col = qlo + tt * 128
                    for co in range(2):
                        po = pspool.tile([128, 512], f32, tag="ps")
                        nc.tensor.matmul(po[:],
                                         yT2_t[:, col:col + 128],
                                         wp_t[:, co * 512:(co + 1) * 512],
                                         start=True, stop=True)
                        so = youtp.tile([128, 512], f16, tag="so")
                        if co == 1 and tt % 2 == 1:
                            nc.scalar.copy(so[:], po[:])
                        else:
                            nc.vector.tensor_copy(so[:], po[:])
                        nc.sync.dma_start(
                            out=out[col:col + 128, co * 512:(co + 1) * 512],
                            in_=so[:])

            def out_proj_final(qlo):
                """Tail-critical variant: both halves per token tile go into
                one [128,1024] score-pool tile (free at this point), one wide
                cast alternating DVE/ScalarE, DMAs on two queues."""
                for tt in range(4):
                    col = qlo + tt * 128
                    po = stps.tile([128, 1024], f32, tag="st")
                    for co in range(2):
                        nc.tensor.matmul(po[:, co * 512:(co + 1) * 512],
                                         yT2_t[:, col:col + 128],
                                         wp_t[:, co * 512:(co + 1) * 512],
                                         start=True, stop=True)
                    so = youtp.tile([128, 1024], f16, tag="sow")
                    if tt % 2 == 0:
                        nc.vector.tensor_copy(so[:], po[:])
                    else:
                        nc.scalar.copy(so[:], po[:])
                    eng = nc.scalar if tt % 2 == 0 else nc.sync
                    eng.dma_start(out=out[col:col + 128, :], in_=so[:])

            # ---------------- pipelined emission ----------------
            # Each attention step interleaves next-chunk projection chains
            # ("fillers") at four points so the PE FIFO always has
            # exp-independent work; the output projection of chunk n is
            # emitted during step n+1 so it never waits on the (DMA-based)
            # softmax-denominator chain.
            def emit_attn(b, qc, fillers, prev_qlo, last=False):
                def run(i):
                    for f in fillers[i::4]:
                        f()
                qlo = b * T + qc * 512
                run(0)
                fin_c = attn_part(b, qc, qlo, is_self=False)
                run(1)
                y_c = fin_c()
                fin_s = attn_part(b, qc, qlo, is_self=True,
                                  mid_hook=lambda: run(2))
                run(3)
                if prev_qlo is not None:
                    out_proj(prev_qlo)
                y_s = fin_s(tail=last)
                combine(b, qc, qlo, y_s, y_c)
                return qlo

            def chunk_fillers(ch):
                return [
                    lambda: chan_chain(ch, wq_t, qT_t, 0),
                    lambda: chan_chain(ch, wk_t, kT_t, 1),
                    lambda: chan_chain(ch, wcq_t, qcT_t, 3),
                    lambda: v_chain(ch),
                ]

            cross_chunk(0)
            for f in chunk_fillers(0):
                f()
            prev = None
            for step in range(8):
                b, qc = divmod(step, QC_PER_B)
                nxt = step + 1
                if nxt < 8:
                    fillers = chunk_fillers(nxt)
                    if nxt == 4:
                        fillers = [lambda: cross_chunk(1)] + fillers
                else:
                    fillers = []
                prev = emit_attn(b, qc, fillers, prev, last=(step == 7))
            out_proj_final(prev)

    nc.compile()
    return nc


_NC_CACHE = {}


def _get_nc(zero_bias=False):
    if zero_bias not in _NC_CACHE:
        _NC_CACHE[zero_bias] = _build(zero_bias)
    return _NC_CACHE[zero_bias]


def warr(w):
    """[C,128] weight -> [128, KT, 128] fp16 (partition-major k-tiles)."""
    kt = w.shape[0] // 128
    return np.ascontiguousarray(
        w.reshape(kt, 128, w.shape[1]).transpose(1, 0, 2)).astype(np.float16)


def make_in_maps(x, cross_input, Wk, bk, Wq, bq, Wv, bv, Wck, bck, Wcq, bcq,
                 Wcv, bcv, Wp, bp):
    """Host-side shard + layout prep. Returns per-core input maps."""
    xT0 = np.asarray(x, np.float32).reshape(NT, C).T.astype(np.float16)  # [C, NT]
    xT = np.ascontiguousarray(
        xT0.reshape(KT_X, 128, NCH, 512).transpose(2, 1, 0, 3))  # [NCH,128,KT,512]
    cT0 = np.asarray(cross_input, np.float32).reshape(NTC, CC).T.astype(np.float16)
    cT = np.ascontiguousarray(
        cT0.reshape(KT_C, 128, NCHC, 512).transpose(2, 1, 0, 3))
    mask = np.triu(np.ones((128, 128), np.float32)).astype(np.float16)
    mask2 = np.ascontiguousarray(np.concatenate([mask, mask], axis=1))
    Wq, Wk, Wv = (np.asarray(w, np.float32) for w in (Wq, Wk, Wv))
    Wcq, Wck, Wcv = (np.asarray(w, np.float32) for w in (Wcq, Wck, Wcv))
    Wp = np.asarray(Wp, np.float32)
    in_maps = []
    for c in range(NCORES):
        sl = slice(c * CPC, (c + 1) * CPC)
        bias8 = np.zeros((CPC, 8), np.float32)
        bias8[:, 0] = np.asarray(bq, np.float32)[sl]
        bias8[:, 1] = np.asarray(bk, np.float32)[sl]
        bias8[:, 2] = np.asarray(bck, np.float32)[sl]
        bias8[:, 3] = np.asarray(bcq, np.float32)[sl]
        bvc = np.asarray(bv, np.float32)[sl] + np.asarray(bcv, np.float32)[sl]
        bias8[0:64, 4] = bvc[0:64]
        bias8[0:64, 5] = bvc[64:128]
        in_maps.append({
            "xT": xT, "cT": cT,
            "wq": warr(Wq[:, sl]), "wk": warr(Wk[:, sl]),
            "wv": warr(Wv[:, sl]), "wcq": warr(Wcq[:, sl]),
            "wck": warr(Wck[:, sl]), "wcv": warr(Wcv[:, sl]),
            "wp": Wp[sl, :].astype(np.float16),
            "bias8": np.ascontiguousarray(bias8),
            "mask2": mask2,
            "ones64": np.ones((128, 64), np.float32),
        })
    return in_maps


def kernel(**inputs):
    in_maps = make_in_maps(**inputs)
    zb = all(not np.any(np.asarray(inputs[k])) for k in
             ("bq", "bk", "bv", "bcq", "bck", "bcv"))
    nc = _get_nc(zero_bias=zb)
    res = run_bass_kernel_spmd(nc, in_maps, list(range(NCORES)))
    acc = np.zeros((NT, C), np.float32)
    for c in range(NCORES):
        acc += res.results[c]["out"].astype(np.float32)
    acc += np.asarray(inputs["bp"], np.float32)
    return acc.reshape(B, T, C).astype(np.float32)


if __name__ == "__main__":
    nc = _get_nc(zero_bias=True)
    print("build + compile OK")


# revision 25
# speedup vs baseline: 1.1676x; 1.1676x over previous
"""Causal self-attention + cross-attention Trainium2 kernel (8 NeuronCores).

Sharding: head-parallel. 16 heads x 2 batches = 32 (b,h) pairs; core c owns
heads {2c, 2c+1} for both batches (its 128 channels of C=1024). Projections
are column-sliced per core; attention runs fully local per head; the output
projection is row-sliced and the 8 partial [B*T, C] fp16 outputs are summed
on the host (no device collectives).

All matmuls are fp16 (1 cycle/row on PE, fp32 PSUM accumulate). Softmax
without max-subtraction (scores bounded ~|8| here), exp on ScalarE with the
1/sqrt(D) scale folded in, scores computed transposed (ST[k,q]) so no
probability transpose is needed before AV. V tiles carry a ones column so
AV row 64 accumulates the softmax denominator.

The whole kernel is a single software-pipelined phase sequence emitted as
  cross-proj(b0), proj(ch0), attn(b0,q0)+out-proj, proj(ch1), attn(b0,q1)...
so projection matmuls fill the PE whenever attention stalls on ScalarE's
exp, the output projection overlaps attention instead of running cold at
the tail, and the PE never idles long enough for HAM to re-throttle.
V is produced token-major directly (x-tile stationary, Wv moving), which
removes the PE-transpose pass and frees its PSUM bank; PSUM budget is
proj/out 2x[128,512] + scores 2x[128,1024] + 2 AV accumulators = 8 banks.
"""
import sys

sys.path.insert(0, "/opt/trn_rl_repo")

import numpy as np

import concourse.bass as bass
import concourse.tile as tile
from concourse import bacc, mybir
from concourse.bass_utils import run_bass_kernel_spmd

dt = mybir.dt

B, T, TC, C, CC, H, D = 2, 2048, 512, 1024, 512, 16, 64
NCORES = 8
CPC = 128          # channels per core = 2 heads * 64
NT = B * T         # 4096 tokens (batch-major)
NTC = B * TC       # 1024 cross tokens
KT_X = C // 128    # 8 contraction tiles over C
KT_C = CC // 128   # 4 contraction tiles over CC
NCH = NT // 512    # 8 token chunks
NCHC = NTC // 512  # 2 cross token chunks
QC_PER_B = T // 512  # 4 q-chunks per batch
KT_PER_B = T // 128  # 16 k-tiles per batch
LOOKAHEAD = 2      # kt steps issued ahead of their AV in the PE queue


def _build(zero_bias=False):
    f32, f16 = dt.float32, dt.float16
    nc = bacc.Bacc("TRN2", target_bir_lowering=False, debug=False,
                   enable_asserts=True, num_devices=NCORES)

    xT = nc.dram_tensor("xT", [NCH, 128, KT_X, 512], f16, kind="ExternalInput").ap()
    cT = nc.dram_tensor("cT", [NCHC, 128, KT_C, 512], f16, kind="ExternalInput").ap()
    wq = nc.dram_tensor("wq", [128, KT_X, CPC], f16, kind="ExternalInput").ap()
    wk = nc.dram_tensor("wk", [128, KT_X, CPC], f16, kind="ExternalInput").ap()
    wv = nc.dram_tensor("wv", [128, KT_X, CPC], f16, kind="ExternalInput").ap()
    wcq = nc.dram_tensor("wcq", [128, KT_X, CPC], f16, kind="ExternalInput").ap()
    wck = nc.dram_tensor("wck", [128, KT_C, CPC], f16, kind="ExternalInput").ap()
    wcv = nc.dram_tensor("wcv", [128, KT_C, CPC], f16, kind="ExternalInput").ap()
    wp = nc.dram_tensor("wp", [CPC, C], f16, kind="ExternalInput").ap()
    bias8 = nc.dram_tensor("bias8", [CPC, 8], f32, kind="ExternalInput").ap()
    maskd = nc.dram_tensor("mask2", [128, 256], f16, kind="ExternalInput").ap()
    onesd = nc.dram_tensor("ones64", [128, 64], dt.float32r, kind="ExternalInput").ap()
    out = nc.dram_tensor("out", [NT, C], f16, kind="ExternalOutput").ap()

    Exp = mybir.ActivationFunctionType.Exp
    SCALE = 0.125  # 1/sqrt(D)

    with tile.TileContext(nc) as tc:
        from contextlib import ExitStack
        with ExitStack() as es:
            persist = es.enter_context(tc.tile_pool(name="persist", bufs=1))
            qT_t = persist.tile([128, NT], f16, tag="qT")
            kT_t = persist.tile([128, NT], f16, tag="kT")
            qcT_t = persist.tile([128, NT], f16, tag="qcT")
            kcT_t = persist.tile([128, NTC], f16, tag="kcT")
            vn_t = persist.tile([128, (NT // 128) * 130], f16, tag="vn")
            vcn_t = persist.tile([128, (NTC // 128) * 130], f16, tag="vcn")
            yT2_t = persist.tile([128, NT], f16, tag="yT2")
            wp_t = persist.tile([128, C], f16, tag="wp")
            bias_t = persist.tile([128, 8], f32, tag="bias")
            mask_t = persist.tile([128, 256], f16, tag="mask")

            wq_t = persist.tile([128, KT_X, CPC], f16, tag="wq")
            wk_t = persist.tile([128, KT_X, CPC], f16, tag="wk")
            wv_t = persist.tile([128, KT_X, CPC], f16, tag="wv")
            wcq_t = persist.tile([128, KT_X, CPC], f16, tag="wcq")
            wck_t = persist.tile([128, KT_C, CPC], f16, tag="wck")
            wcv_t = persist.tile([128, KT_C, CPC], f16, tag="wcv")

            # critical-path loads first, split across engine DMA queues:
            # scalar carries weights, sync carries x/cross chunks (in-loop),
            # gpsimd/vector carry tensors not needed until later.
            for wdram, wtile in ((wck, wck_t), (wcv, wcv_t), (wq, wq_t),
                                 (wk, wk_t), (wv, wv_t), (wcq, wcq_t)):
                nc.scalar.dma_start(out=wtile[:], in_=wdram[:])
            nc.gpsimd.dma_start(out=bias_t[:], in_=bias8[:])
            nc.gpsimd.dma_start(out=mask_t[:], in_=maskd[:])
            nc.gpsimd.dma_start(out=wp_t[:], in_=wp[:])

            vn_r = vn_t[:].rearrange("p (t c) -> p t c", c=130)
            nc.vector.memset(vn_r[:, :, 64:65], 1.0)
            nc.vector.memset(vn_r[:, :, 129:130], 1.0)
            vcn_r = vcn_t[:].rearrange("p (t c) -> p t c", c=130)
            nc.vector.memset(vcn_r[:, :, 64:65], 1.0)
            nc.vector.memset(vcn_r[:, :, 129:130], 1.0)
            ones_t = persist.tile([128, 64], dt.float32r, tag="ones")
            nc.gpsimd.dma_start(out=ones_t[:], in_=onesd[:])

            apool = es.enter_context(tc.tile_pool(name="apool", bufs=3))
            bpool = es.enter_context(tc.tile_pool(name="bpool", bufs=3))
            rdpool = es.enter_context(tc.tile_pool(name="rdpool", bufs=2, space="DRAM"))
            ypool = es.enter_context(tc.tile_pool(name="ypool", bufs=4))
            expool = es.enter_context(tc.tile_pool(name="expool", bufs=12))
            youtp = es.enter_context(tc.tile_pool(name="youtp", bufs=6))
            pspool = es.enter_context(tc.tile_pool(name="pspool", bufs=2, space="PSUM"))
            stps = es.enter_context(tc.tile_pool(name="stps", bufs=2, space="PSUM"))
            yhps = es.enter_context(tc.tile_pool(name="yhps", bufs=1, space="PSUM"))

            def evict(dst_slice, ps, bcol):
                if zero_bias:
                    nc.vector.tensor_copy(dst_slice, ps[:])
                else:
                    nc.vector.tensor_scalar_add(dst_slice, ps[:],
                                                bias_t[:, bcol:bcol + 1])

            xblks = {}

            def get_xblk(ch):
                if ch not in xblks:
                    xblk = apool.tile([128, KT_X, 512], f16, tag="xblk")
                    nc.sync.dma_start(out=xblk[:], in_=xT[ch])
                    xblks[ch] = xblk
                return xblks[ch]

            def chan_chain(ch, wtile, dst, bcol):
                """One channel-major projection chain for a 512-token chunk."""
                xblk = get_xblk(ch)
                ps = pspool.tile([128, 512], f32, tag="ps")
                for kt in range(KT_X):
                    nc.tensor.matmul(ps[:], wtile[:, kt, :], xblk[:, kt, :],
                                     start=(kt == 0), stop=(kt == KT_X - 1))
                evict(dst[:, ch * 512:(ch + 1) * 512], ps, bcol)

            def v_chain(ch):
                """v token-major: out[tok, d] accumulated per 128-tok subtile."""
                xblk = get_xblk(ch)
                pv = pspool.tile([128, 512], f32, tag="ps")
                for tt in range(4):
                    for kt in range(KT_X):
                        nc.tensor.matmul(
                            pv[:, tt * 128:(tt + 1) * 128],
                            xblk[:, kt, tt * 128:(tt + 1) * 128],
                            wv_t[:, kt, :],
                            start=(kt == 0), stop=(kt == KT_X - 1))
                # single strided copy into vn [v_h0 | 1 | v_h1 | 1] slots
                base = ch * 4 * 130
                dst = vn_t[:, base:base + 4 * 130].rearrange(
                    "p (t h c) -> p t h c", t=4, h=2)[:, :, :, 0:64]
                src = pv[:].rearrange("p (t h c) -> p t h c", t=4, h=2)
                nc.vector.tensor_copy(dst, src)

            def cross_chunk(chc):
                """kc projection (channel-major) + vc (token-major) for one
                512-token cross chunk."""
                cblk = apool.tile([128, KT_C, 512], f16, tag="cblk", bufs=2)
                nc.sync.dma_start(out=cblk[:], in_=cT[chc])
                ps = pspool.tile([128, 512], f32, tag="ps")
                for kt in range(KT_C):
                    nc.tensor.matmul(ps[:], wck_t[:, kt, :], cblk[:, kt, :],
                                     start=(kt == 0), stop=(kt == KT_C - 1))
                evict(kcT_t[:, chc * 512:(chc + 1) * 512], ps, 2)
                pv = pspool.tile([128, 512], f32, tag="ps")
                for tt in range(4):
                    for kt in range(KT_C):
                        nc.tensor.matmul(
                            pv[:, tt * 128:(tt + 1) * 128],
                            cblk[:, kt, tt * 128:(tt + 1) * 128],
                            wcv_t[:, kt, :],
                            start=(kt == 0), stop=(kt == KT_C - 1))
                base = chc * 4 * 130
                dst = vcn_t[:, base:base + 4 * 130].rearrange(
                    "p (t h c) -> p t h c", t=4, h=2)[:, :, :, 0:64]
                src = pv[:].rearrange("p (t h c) -> p t h c", t=4, h=2)
                nc.vector.tensor_copy(dst, src)

            mask_r = mask_t[:].rearrange("p (h q) -> p h q", h=2)

            def attn_part(b, qc, qlo, is_self, mid_hook=None):
                """One softmax-attention accumulation (self or cross) for a
                512-wide q chunk of batch b. Both heads' transposed score
                tiles live in one [128,1024] two-bank PSUM tile so a single
                exp serves both. Returns per-head normalized [64,512]."""
                nkt = (4 * qc + 4) if is_self else KT_C
                mid_kt = nkt // 2
                yh = yhps.tile([65, 1024], f32, tag="yh")

                pend = []
                fidx = [0]

                def flush_one():
                    ex, off, vsrc, vc0, vc1 = pend.pop(0)
                    first = fidx[0] == 0
                    last = fidx[0] == nkt - 1
                    fidx[0] += 1
                    nc.tensor.matmul(
                        yh[0:65, off:512],
                        vsrc[:, vc0:vc0 + 65],
                        ex[:, off:512],
                        start=first, stop=last)
                    nc.tensor.matmul(
                        yh[0:65, 512 + off:1024],
                        vsrc[:, vc1:vc1 + 65],
                        ex[:, 512 + off:1024],
                        start=first, stop=last)

                for kt in range(nkt):
                    if mid_hook is not None and kt == mid_kt:
                        mid_hook()
                    if is_self:
                        crossing = kt >= 4 * qc
                        off = (kt - 4 * qc) * 128 if crossing else 0
                        klo = b * T + kt * 128
                        ksrc, qsrc, vsrc = kT_t, qT_t, vn_t
                        vbase = (b * KT_PER_B + kt) * 130
                    else:
                        crossing, off = False, 0
                        klo = b * TC + kt * 128
                        ksrc, qsrc, vsrc = kcT_t, qcT_t, vcn_t
                        vbase = (b * KT_C + kt) * 130
                    st = stps.tile([128, 1024], f32, tag="st")
                    nc.tensor.matmul(
                        st[:, off:512],
                        ksrc[0:64, klo:klo + 128],
                        qsrc[0:64, qlo + off:qlo + 512],
                        start=True, stop=True)
                    nc.tensor.matmul(
                        st[:, 512 + off:1024],
                        ksrc[64:128, klo:klo + 128],
                        qsrc[64:128, qlo + off:qlo + 512],
                        start=True, stop=True)
                    ex = expool.tile([128, 1024], f16, tag="ex")
                    if off == 0:
                        nc.scalar.activation(ex[:], st[:], Exp, scale=SCALE)
                    else:
                        ex3 = ex[:].rearrange("p (h q) -> p h q", h=2)
                        st3 = st[:].rearrange("p (h q) -> p h q", h=2)
                        nc.scalar.activation(ex3[:, :, off:512],
                                             st3[:, :, off:512], Exp, scale=SCALE)
                    if crossing:
                        ex3 = ex[:].rearrange("p (h q) -> p h q", h=2)
                        nc.gpsimd.tensor_mul(ex3[:, :, off:off + 128],
                                             ex3[:, :, off:off + 128], mask_r)
                    pend.append((ex, off, vsrc, vbase, vbase + 65))
                    if len(pend) > LOOKAHEAD:
                        flush_one()
                while pend:
                    flush_one()

                def finish(tail=False):
                    # Evict accumulators (incl. the denominator row 64) to
                    # SBUF, get 1/denominator broadcast to 64 partitions,
                    # then one wide multiply on DVE. Mid-stream parts use a
                    # DMA bounce through DRAM (compact reciprocal on
                    # [128,8], broadcast-read back) — fully hidden behind
                    # ScalarE/PE. The tail-exposed final part instead
                    # broadcasts with two rank-1 f32r matmuls (PE is idle
                    # there) + fast approx reciprocal, which is ~6us
                    # shorter in latency.
                    ysb = bpool.tile([65, 1024], f32, tag="ysb")
                    nc.vector.tensor_copy(ysb[:], yh[:])
                    if tail:
                        dsr = bpool.tile([65, 1024], dt.float32r, tag="dsr")
                        nc.vector.tensor_copy(dsr[64:65, :], yh[64:65, :])
                        pbA = pspool.tile([64, 512], f32, tag="ps")
                        nc.tensor.matmul(pbA[:], ones_t[64:65, :],
                                         dsr[64:65, 0:512],
                                         start=True, stop=True)
                        pbB = pspool.tile([64, 512], f32, tag="ps")
                        nc.tensor.matmul(pbB[:], ones_t[64:65, :],
                                         dsr[64:65, 512:1024],
                                         start=True, stop=True)
                        r = bpool.tile([64, 1024], f32, tag="bc")
                        nc.vector.reciprocal_approx_fast(r[:, 0:512], pbA[:])
                        nc.vector.reciprocal_approx_fast(r[:, 512:1024],
                                                         pbB[:])
                    else:
                        drd = rdpool.tile([1, 1024], f32, tag="drd")
                        nc.sync.dma_start(out=drd[:], in_=ysb[64:65, :])
                        dsb = bpool.tile([128, 8], f32, tag="dsb")
                        nc.sync.dma_start(
                            out=dsb[:],
                            in_=drd[:].rearrange("a (p e) -> p (a e)", p=128))
                        rsb = bpool.tile([128, 8], f32, tag="rsb")
                        nc.vector.reciprocal(rsb[:], dsb[:])
                        rrd = rdpool.tile([1, 1024], f32, tag="rrd")
                        nc.sync.dma_start(
                            out=rrd[:].rearrange("a (p e) -> p (a e)", p=128),
                            in_=rsb[:])
                        r = bpool.tile([64, 1024], f32, tag="bc")
                        nc.sync.dma_start(
                            out=r[:, 0:512],
                            in_=rrd[0:1, 0:512].to_broadcast((64, 512)))
                        nc.sync.dma_start(
                            out=r[:, 512:1024],
                            in_=rrd[0:1, 512:1024].to_broadcast((64, 512)))
                    yab = ypool.tile([64, 1024], f32, tag="yab")
                    nc.vector.tensor_mul(yab[:], ysb[0:64, :], r[:])
                    return yab

                return finish

            def combine(b, qc, qlo, y_s, y_c):
                if zero_bias:
                    nc.vector.tensor_add(yT2_t[0:64, qlo:qlo + 512],
                                         y_s[:, 0:512], y_c[:, 0:512])
                    ybsum = ypool.tile([64, 512], f16, tag="ybsum")
                    nc.vector.tensor_add(ybsum[:], y_s[:, 512:1024],
                                         y_c[:, 512:1024])
                else:
                    add = mybir.AluOpType.add
                    nc.vector.scalar_tensor_tensor(
                        yT2_t[0:64, qlo:qlo + 512], y_s[:, 0:512],
                        bias_t[0:64, 4:5], y_c[:, 0:512], add, add)
                    ybsum = ypool.tile([64, 512], f16, tag="ybsum")
                    nc.vector.scalar_tensor_tensor(
                        ybsum[:], y_s[:, 512:1024], bias_t[0:64, 5:6],
                        y_c[:, 512:1024], add, add)
                # partition shift rows 0-63 -> 64-127 via SBUF-SBUF DMA
                nc.sync.dma_start(out=yT2_t[64:128, qlo:qlo + 512],
                                  in_=ybsum[:])

            def out_proj(qlo):
                """Output projection for one 512-token chunk of yT2.
                Evictions mostly on DVE; 1 in 4 on ScalarE (which has slack)."""
                for tt in range(4):
                    col = qlo + tt * 128
                    for co in range(2):
                        po = pspool.tile([128, 512], f32, tag="ps")
                        nc.tensor.matmul(po[:],
                                         yT2_t[:, col:col + 128],
                                         wp_t[:, co * 512:(co + 1) * 512],
                                         start=True, stop=True)
                        so = youtp.tile([128, 512], f16, tag="so")
                        if co == 1 and tt % 2 == 1:
                            nc.scalar.copy(so[:], po[:])
                        else:
                            nc.vector.tensor_copy(so[:], po[:])
                        nc.gpsimd.dma_start(
                            out=out[col:col + 128, co * 512:(co + 1) * 512],
                            in_=so[:])

            def out_proj_final(qlo):
                """Tail-critical variant: both halves per token tile go into
                one [128,1024] score-pool tile (free at this point), one wide
                cast alternating DVE/ScalarE, DMAs on two queues."""
                for tt in range(4):
                    col = qlo + tt * 128
                    po = stps.tile([128, 1024], f32, tag="st")
                    for co in range(2):
                        nc.tensor.matmul(po[:, co * 512:(co + 1) * 512],
                                         yT2_t[:, col:col + 128],
                                         wp_t[:, co * 512:(co + 1) * 512],
                                         start=True, stop=True)
                    so = youtp.tile([128, 1024], f16, tag="sow")
                    if tt % 2 == 0:
                        nc.vector.tensor_copy(so[:], po[:])
                    else:
                        nc.scalar.copy(so[:], po[:])
                    eng = nc.scalar if tt % 2 == 0 else nc.sync
                    eng.dma_start(out=out[col:col + 128, :], in_=so[:])

            # ---------------- pipelined emission ----------------
            # Each attention step interleaves next-chunk projection chains
            # ("fillers") at four points so the PE FIFO always has
            # exp-independent work; the output projection of chunk n is
            # emitted during step n+1 so it never waits on the (DMA-based)
            # softmax-denominator chain.
            def emit_attn(b, qc, fillers, prev_qlo, last=False):
                def run(i):
                    for f in fillers[i::4]:
                        f()
                qlo = b * T + qc * 512
                run(0)
                fin_c = attn_part(b, qc, qlo, is_self=False)
                run(1)
                y_c = fin_c()
                fin_s = attn_part(b, qc, qlo, is_self=True,
                                  mid_hook=lambda: run(2))
                run(3)
                if prev_qlo is not None:
                    out_proj(prev_qlo)
                y_s = fin_s(tail=last)
                combine(b, qc, qlo, y_s, y_c)
                return qlo

            def chunk_fillers(ch):
                return [
                    lambda: chan_chain(ch, wq_t, qT_t, 0),
                    lambda: chan_chain(ch, wk_t, kT_t, 1),
                    lambda: chan_chain(ch, wcq_t, qcT_t, 3),
                    lambda: v_chain(ch),
                ]

            cross_chunk(0)
            for f in chunk_fillers(0):
                f()
            prev = None
            for step in range(8):
                b, qc = divmod(step, QC_PER_B)
                nxt = step + 1
                if nxt < 8:
                    fillers = chunk_fillers(nxt)
                    if nxt == 4:
                        fillers = [lambda: cross_chunk(1)] + fillers
                else:
                    fillers = []
                prev = emit_attn(b, qc, fillers, prev, last=(step == 7))
            out_proj_final(prev)

    nc.compile()
    return nc


_NC_CACHE = {}


def _get_nc(zero_bias=False):
    if zero_bias not in _NC_CACHE:
        _NC_CACHE[zero_bias] = _build(zero_bias)
    return _NC_CACHE[zero_bias]


def warr(w):
    """[C,128] weight -> [128, KT, 128] fp16 (partition-major k-tiles)."""
    kt = w.shape[0] // 128
    return np.ascontiguousarray(
        w.reshape(kt, 128, w.shape[1]).transpose(1, 0, 2)).astype(np.float16)


def make_in_maps(x, cross_input, Wk, bk, Wq, bq, Wv, bv, Wck, bck, Wcq, bcq,
                 Wcv, bcv, Wp, bp):
    """Host-side shard + layout prep. Returns per-core input maps."""
    xT0 = np.asarray(x, np.float32).reshape(NT, C).T.astype(np.float16)  # [C, NT]
    xT = np.ascontiguousarray(
        xT0.reshape(KT_X, 128, NCH, 512).transpose(2, 1, 0, 3))  # [NCH,128,KT,512]
    cT0 = np.asarray(cross_input, np.float32).reshape(NTC, CC).T.astype(np.float16)
    cT = np.ascontiguousarray(
        cT0.reshape(KT_C, 128, NCHC, 512).transpose(2, 1, 0, 3))
    mask = np.triu(np.ones((128, 128), np.float32)).astype(np.float16)
    mask2 = np.ascontiguousarray(np.concatenate([mask, mask], axis=1))
    Wq, Wk, Wv = (np.asarray(w, np.float32) for w in (Wq, Wk, Wv))
    Wcq, Wck, Wcv = (np.asarray(w, np.float32) for w in (Wcq, Wck, Wcv))
    Wp = np.asarray(Wp, np.float32)
    in_maps = []
    for c in range(NCORES):
        sl = slice(c * CPC, (c + 1) * CPC)
        bias8 = np.zeros((CPC, 8), np.float32)
        bias8[:, 0] = np.asarray(bq, np.float32)[sl]
        bias8[:, 1] = np.asarray(bk, np.float32)[sl]
        bias8[:, 2] = np.asarray(bck, np.float32)[sl]
        bias8[:, 3] = np.asarray(bcq, np.float32)[sl]
        bvc = np.asarray(bv, np.float32)[sl] + np.asarray(bcv, np.float32)[sl]
        bias8[0:64, 4] = bvc[0:64]
        bias8[0:64, 5] = bvc[64:128]
        in_maps.append({
            "xT": xT, "cT": cT,
            "wq": warr(Wq[:, sl]), "wk": warr(Wk[:, sl]),
            "wv": warr(Wv[:, sl]), "wcq": warr(Wcq[:, sl]),
            "wck": warr(Wck[:, sl]), "wcv": warr(Wcv[:, sl]),
            "wp": Wp[sl, :].astype(np.float16),
            "bias8": np.ascontiguousarray(bias8),
            "mask2": mask2,
            "ones64": np.ones((128, 64), np.float32),
        })
    return in_maps


def kernel(**inputs):
    in_maps = make_in_maps(**inputs)
    zb = all(not np.any(np.asarray(inputs[k])) for k in
             ("bq", "bk", "bv", "bcq", "bck", "bcv"))
    nc = _get_nc(zero_bias=zb)
    res = run_bass_kernel_spmd(nc, in_maps, list(range(NCORES)))
    acc = np.zeros((NT, C), np.float32)
    for c in range(NCORES):
        acc += res.results[c]["out"].astype(np.float32)
    acc += np.asarray(inputs["bp"], np.float32)
    return acc.reshape(B, T, C).astype(np.float32)


if __name__ == "__main__":
    nc = _get_nc(zero_bias=True)
    print("build + compile OK")
